# revision 4
# baseline (speedup 1.0000x reference)
"""Trainium2 Bass kernel for a TransformerEncoderLayer self-attention block.

Contract: kernel(**inputs) takes the FULL unsharded inputs
    src        [2048, 8, 512] f32
    in_proj_w  [1536, 512]    f32
    in_proj_b  [1536]         f32
    out_w      [512, 512]     f32
    out_b      [512]          f32
and returns (out [2048, 8, 512] f32, attn_weights [8, 2048, 2048] f32),
matching the reference nn.Module (packed QKV -> 8-head SDPA -> out proj,
plus head-averaged attention weights).

Sharding: pure data parallel over the batch dim — batch b -> NeuronCore b
(8 batches, 8 cores). Each core runs the identical single-batch kernel.

Per-core dataflow (S=2048, D=512, H=8, HD=64):
  phase 0: PE-transpose src, in_proj_w, out_w into contraction-major
           layouts (srcT [D,S], winT [D,3D], woT [D,D]).
  phase 1: qkvT [3D, S] = winT.T @ srcT on PE (fp32r), bias added on DVE
           during PSUM evacuation.  q/k kept fp32r, v cast to bf16.
  phase 2: per s-chunk of 512, per head:
           scoresT [t,s] on PE (K=64, head pairs placed at partition
           offsets 0/64 so row-tiled matmuls overlap);
           exp on ACT straight out of PSUM (scale=1/8 fused) -> bf16;
           ctx + rowsum via one PE matmul against [v | 1];
           attn-average path: PE transposes exp tiles back to [s,t] and a
           DVE scalar_tensor_tensor accumulates c_h[s]*exp into fp16;
           out = ctxT.T @ woT + bias on PE/DVE.
"""

import numpy as np

S, B, D, H = 2048, 8, 512, 8
HD = D // H  # 64

_CACHE = {}


def _build_nc(qk_dtype_name: str):
    import concourse.bass as bass  # noqa: F401
    import concourse.tile as tile
    from concourse import bacc, mybir
    from concourse.masks import make_identity

    F32 = mybir.dt.float32
    F32R = mybir.dt.float32r
    BF16 = mybir.dt.bfloat16
    FP16 = mybir.dt.float16
    QK = {"f32r": F32R, "bf16": BF16}[qk_dtype_name]
    AF = mybir.ActivationFunctionType
    OP = mybir.AluOpType

    nc = bacc.Bacc("TRN2", target_bir_lowering=False, debug=False, num_devices=B)

    src = nc.dram_tensor("src", [S, D], F32, kind="ExternalInput")
    win = nc.dram_tensor("in_proj_w", [3 * D, D], F32, kind="ExternalInput")
    bin_ = nc.dram_tensor("in_proj_b", [3 * D], F32, kind="ExternalInput")
    wo = nc.dram_tensor("out_w", [D, D], F32, kind="ExternalInput")
    bo = nc.dram_tensor("out_b", [D], F32, kind="ExternalInput")
    out = nc.dram_tensor("out", [S, D], F32, kind="ExternalOutput")
    attn = nc.dram_tensor("attn", [S, S], F32, kind="ExternalOutput")

    NS = S // 128  # 16 s/t tiles
    NC_ = 4        # s-chunks of 512
    ST_PER_C = 4   # s-tiles per chunk
    NT = 16        # t tiles

    with tile.TileContext(nc) as tc:
        with (
            tc.tile_pool(name="consts", bufs=1) as consts,
            tc.tile_pool(name="qk", bufs=1) as qk_pool,
            tc.tile_pool(name="vext", bufs=1) as vext_pool,
            tc.tile_pool(name="wacc", bufs=4) as wacc_pool,
            tc.tile_pool(name="persist", bufs=1) as persist,
        ):
            # ---------------- constants ----------------
            ident_bf = consts.tile([128, 128], BF16, tag="ident_bf")
            make_identity(nc, ident_bf)
            ident_f32 = consts.tile([128, 128], F32, tag="ident_f32")
            make_identity(nc, ident_f32)

            bin_sb = consts.tile([128, 12], F32, tag="bin")
            # in_proj_b[128*j + p] -> bin_sb[p, j]
            nc.gpsimd.dma_start(
                out=bin_sb,
                in_=bass.AP(tensor=bin_, offset=0, ap=[[1, 128], [128, 12]]),
            )
            bo_bcast = consts.tile([128, D], F32, tag="bo")
            nc.gpsimd.dma_start(
                out=bo_bcast,
                in_=bass.AP(tensor=bo, offset=0, ap=[[0, 128], [1, D]]),
            )

            # persistent activations
            qkT = qk_pool.tile([128, 8, S], QK, tag="qkT")      # q rows 0-511, k rows 512-1023
            vext = vext_pool.tile([128, H, NT, HD + 1], BF16, tag="vext")
            ctx_sb = persist.tile([128, ST_PER_C, D], BF16, tag="ctx")
            ctxT_sb = persist.tile([128, 4, 512], BF16, tag="ctxT")
            woT = persist.tile([128, 4, D], BF16, tag="woT")

            # ---------------- phase 0: transposes ----------------
            with (
                tc.tile_pool(name="stage", bufs=2) as stage_pool,
                tc.tile_pool(name="stageT", bufs=1) as stageT_pool,
                tc.tile_pool(name="ps0", bufs=2, space="PSUM") as ps0,
            ):
                srcT = stageT_pool.tile([128, 4, S], F32R, tag="srcT")
                winT = stageT_pool.tile([128, 4, 3 * D], F32R, tag="winT")
                vT = stageT_pool.tile([128, 4, S], BF16, tag="vT")

                # src [S, D] -> srcT[p, kk, s] = src[s, 128*kk + p]
                for stg in range(4):  # groups of 4 s-tiles
                    st_sb = stage_pool.tile([128, 4, D], F32, tag="stage")
                    nc.sync.dma_start(
                        out=st_sb, in_=src[128 * 4 * stg : 128 * 4 * (stg + 1), :]
                        .rearrange("(a p) d -> p a d", p=128)
                    )
                    for kk in range(4):
                        pt = ps0.tile([128, 512], F32, tag="pt")
                        for j in range(4):
                            nc.tensor.transpose(
                                pt[:, 128 * j : 128 * (j + 1)],
                                st_sb[:, j, 128 * kk : 128 * (kk + 1)],
                                ident_f32,
                            )
                        nc.vector.tensor_copy(
                            srcT[:, kk, 512 * stg : 512 * (stg + 1)], pt
                        )

                # win [1536, 512] -> winT[p, kk, dd] = win[dd, 128*kk + p]
                for jg in range(3):  # groups of 4 dd-tiles
                    st_sb = stage_pool.tile([128, 4, D], F32, tag="stage")
                    nc.sync.dma_start(
                        out=st_sb, in_=win[128 * 4 * jg : 128 * 4 * (jg + 1), :]
                        .rearrange("(a p) d -> p a d", p=128)
                    )
                    for kk in range(4):
                        pt = ps0.tile([128, 512], F32, tag="pt")
                        for j in range(4):
                            nc.tensor.transpose(
                                pt[:, 128 * j : 128 * (j + 1)],
                                st_sb[:, j, 128 * kk : 128 * (kk + 1)],
                                ident_f32,
                            )
                        nc.vector.tensor_copy(
                            winT[:, kk, 512 * jg : 512 * (jg + 1)], pt
                        )

                # wo [512, 512] -> woT[p, kk, d'] = wo[d', 128*kk + p]  (bf16)
                st_sb = stage_pool.tile([128, 4, D], F32, tag="stage")
                nc.sync.dma_start(
                    out=st_sb, in_=wo[:, :].rearrange("(a p) d -> p a d", p=128)
                )
                for kk in range(4):
                    pt = ps0.tile([128, 512], F32, tag="pt")
                    for j in range(4):
                        nc.tensor.transpose(
                            pt[:, 128 * j : 128 * (j + 1)],
                            st_sb[:, j, 128 * kk : 128 * (kk + 1)],
                            ident_f32,
                        )
                    nc.scalar.copy(woT[:, kk, :], pt)

                # ---------------- phase 1: qkvT ----------------
                for j in range(12):
                    for sc in range(4):
                        pq = ps0.tile([128, 512], F32, tag="pq")
                        for kk in range(4):
                            nc.tensor.matmul(
                                pq,
                                winT[:, kk, 128 * j : 128 * (j + 1)],
                                srcT[:, kk, 512 * sc : 512 * (sc + 1)],
                                start=(kk == 0),
                                stop=(kk == 3),
                            )
                        if j < 8:
                            dst = qkT[:, j, 512 * sc : 512 * (sc + 1)]
                        else:
                            dst = vT[:, j - 8, 512 * sc : 512 * (sc + 1)]
                        nc.vector.tensor_scalar_add(dst, pq, bin_sb[:, j : j + 1])

                # ---------------- phase 1.5: vext = [v | 1] per head ----------------
                nc.vector.memset(vext[:, :, :, HD : HD + 1], 1.0)
                for h in range(H):
                    p0 = 64 * (h % 2)
                    jj = h // 2
                    for tg in range(4):  # groups of 4 t-tiles
                        pv = ps0.tile([128, 4, HD], BF16, tag="pv")
                        for j in range(4):
                            tt = 4 * tg + j
                            nc.tensor.transpose(
                                pv[:, j, :],
                                vT[p0 : p0 + 64, jj, 128 * tt : 128 * (tt + 1)],
                                ident_bf[p0 : p0 + 64, p0 : p0 + 64],
                            )
                        nc.vector.tensor_copy(
                            vext[:, h, 4 * tg : 4 * (tg + 1), 0:HD], pv
                        )

            # ---------------- phase 2: attention ----------------
            with (
                tc.tile_pool(name="expT", bufs=3) as exp_pool,
                tc.tile_pool(name="scps", bufs=2, space="PSUM") as sc_ps,
                tc.tile_pool(name="ctxps", bufs=2, space="PSUM") as ctx_ps,
                tc.tile_pool(name="wps", bufs=2, space="PSUM") as w_ps,
                tc.tile_pool(name="small", bufs=8) as small,
                tc.tile_pool(name="wstage", bufs=2) as wstage_pool,
                tc.tile_pool(name="ostage", bufs=2) as ostage_pool,
            ):
                for c in range(NC_):  # s-chunk of 512
                    accs = []
                    for st in range(ST_PER_C):
                        acc_t = wacc_pool.tile([128, S], FP16, tag="acc", name=f"acc_{c}_{st}")
                        accs.append(acc_t)
                    for hp in range(H // 2):
                        pair = (2 * hp, 2 * hp + 1)
                        exps = {}
                        for h in pair:
                            exps[h] = exp_pool.tile(
                                [128, NT, 512], BF16, tag="expT", name=f"expT_{c}_{h}"
                            )
                        # scoresT + exp, head-pair interleaved
                        for g in range(NT // 2):
                            for h in pair:
                                p0 = 64 * (h % 2)
                                jq = h // 2
                                jk = 4 + h // 2
                                scp = sc_ps.tile([128, 2, 512], F32, tag="scp")
                                for i in range(2):
                                    tt = 2 * g + i
                                    nc.tensor.matmul(
                                        scp[:, i, :],
                                        qkT[p0 : p0 + 64, jk, 128 * tt : 128 * (tt + 1)],
                                        qkT[p0 : p0 + 64, jq, 512 * c : 512 * (c + 1)],
                                        start=True,
                                        stop=True,
                                    )
                                nc.scalar.activation(
                                    exps[h][:, 2 * g : 2 * (g + 1), :],
                                    scp,
                                    AF.Exp,
                                    bias=0.0,
                                    scale=float(HD) ** -0.5,
                                )
                        # ctx + rowsum + attn-average accumulation
                        for h in pair:
                            eT = exps[h]
                            for st in range(ST_PER_C):
                                s_lo = 128 * st
                                ctxp = ctx_ps.tile([128, 512], F32, tag="ctxp")
                                for tt in range(NT):
                                    nc.tensor.matmul(
                                        ctxp[:, 0 : HD + 1],
                                        eT[:, tt, s_lo : s_lo + 128],
                                        vext[:, h, tt, :],
                                        start=(tt == 0),
                                        stop=(tt == NT - 1),
                                    )
                                r = small.tile([128, 1], F32, tag="recip")
                                nc.vector.reciprocal(r, ctxp[:, HD : HD + 1])
                                ws = small.tile([128, 1], F32, tag="wscale")
                                nc.vector.tensor_scalar_mul(ws, r, 1.0 / H)
                                nc.vector.tensor_scalar(
                                    ctx_sb[:, st, HD * h : HD * (h + 1)],
                                    ctxp[:, 0:HD],
                                    r[:, 0:1],
                                    None,
                                    OP.mult,
                                )
                                # transpose exp back to [s, t'] and accumulate
                                for half in range(2):
                                    wp = w_ps.tile([128, 8, 128], BF16, tag="wp")
                                    for j in range(8):
                                        tt = 8 * half + j
                                        nc.tensor.transpose(
                                            wp[:, j, :],
                                            eT[:, tt, s_lo : s_lo + 128],
                                            ident_bf,
                                        )
                                    acc_sl = accs[st][:, 1024 * half : 1024 * (half + 1)]
                                    wp_flat = wp.rearrange("p a b -> p (a b)")
                                    if h == 0:
                                        nc.vector.tensor_scalar(
                                            acc_sl, wp_flat, ws[:, 0:1], None, OP.mult
                                        )
                                    else:
                                        nc.vector.scalar_tensor_tensor(
                                            acc_sl, wp_flat, ws[:, 0:1], acc_sl,
                                            OP.mult, OP.add,
                                        )

                    # finalize chunk: attn rows out, ctxT, out-proj
                    for st in range(ST_PER_C):
                        wf = wstage_pool.tile([128, S], F32, tag="wstage")
                        nc.vector.tensor_copy(wf, accs[st])
                        nc.sync.dma_start(
                            out=attn[128 * (ST_PER_C * c + st) : 128 * (ST_PER_C * c + st + 1), :],
                            in_=wf,
                        )
                        # ctxT: [s, d] -> [d, s] for out-proj lhsT
                        wp = w_ps.tile([128, 8, 128], BF16, tag="wp")
                        for dt in range(4):
                            nc.tensor.transpose(
                                wp[:, dt, :],
                                ctx_sb[:, st, 128 * dt : 128 * (dt + 1)],
                                ident_bf,
                            )
                        nc.vector.tensor_copy(
                            ctxT_sb[:, :, 128 * st : 128 * (st + 1)],
                            wp[:, 0:4, :],
                        )
                    for st in range(ST_PER_C):
                        outp = ctx_ps.tile([128, 512], F32, tag="ctxp")
                        for dt in range(4):
                            nc.tensor.matmul(
                                outp,
                                ctxT_sb[:, dt, 128 * st : 128 * (st + 1)],
                                woT[:, dt, :],
                                start=(dt == 0),
                                stop=(dt == 3),
                            )
                        ost = ostage_pool.tile([128, D], F32, tag="ostage")
                        nc.vector.tensor_tensor(ost, outp, bo_bcast, OP.add)
                        nc.sync.dma_start(
                            out=out[128 * (ST_PER_C * c + st) : 128 * (ST_PER_C * c + st + 1), :],
                            in_=ost,
                        )

    nc.compile()
    return nc


def _get_nc(qk_dtype_name="f32r"):
    key = qk_dtype_name
    if key not in _CACHE:
        _CACHE[key] = _build_nc(qk_dtype_name)
    return _CACHE[key]


def kernel(src, in_proj_w, in_proj_b, out_w, out_b):
    from concourse.bass_utils import run_bass_kernel_spmd

    src = np.ascontiguousarray(np.asarray(src, dtype=np.float32))
    in_proj_w = np.ascontiguousarray(np.asarray(in_proj_w, dtype=np.float32))
    in_proj_b = np.ascontiguousarray(np.asarray(in_proj_b, dtype=np.float32))
    out_w = np.ascontiguousarray(np.asarray(out_w, dtype=np.float32))
    out_b = np.ascontiguousarray(np.asarray(out_b, dtype=np.float32))

    nc = _get_nc()
    in_maps = [
        {
            "src": np.ascontiguousarray(src[:, b, :]),
            "in_proj_w": in_proj_w,
            "in_proj_b": in_proj_b,
            "out_w": out_w,
            "out_b": out_b,
        }
        for b in range(B)
    ]
    res = run_bass_kernel_spmd(nc, in_maps, core_ids=list(range(B)))
    out = np.empty((S, B, D), dtype=np.float32)
    attn = np.empty((B, S, S), dtype=np.float32)
    for b in range(B):
        out[:, b, :] = res.results[b]["out"]
        attn[b] = res.results[b]["attn"]
    return out, attn


if __name__ == "__main__":
    rng = np.random.default_rng(0)
    ins = {
        "src": rng.standard_normal((S, B, D), dtype=np.float32),
        "in_proj_w": rng.standard_normal((3 * D, D), dtype=np.float32) * D**-0.5,
        "in_proj_b": rng.standard_normal((3 * D,), dtype=np.float32) * 0.02,
        "out_w": rng.standard_normal((D, D), dtype=np.float32) * D**-0.5,
        "out_b": rng.standard_normal((D,), dtype=np.float32) * 0.02,
    }
    o, a = kernel(**ins)
    print("out", o.shape, "attn", a.shape)


# revision 9
# speedup vs baseline: 1.1384x; 1.1384x over previous
"""Trainium2 Bass kernel for a TransformerEncoderLayer self-attention block.

Contract: kernel(**inputs) takes the FULL unsharded inputs
    src        [2048, 8, 512] f32
    in_proj_w  [1536, 512]    f32
    in_proj_b  [1536]         f32
    out_w      [512, 512]     f32
    out_b      [512]          f32
and returns (out [2048, 8, 512] f32, attn_weights [8, 2048, 2048] f32),
matching the reference nn.Module (packed QKV -> 8-head SDPA -> out proj,
plus head-averaged attention weights).

Sharding: pure data parallel over the batch dim — batch b -> NeuronCore b
(8 batches, 8 cores). Each core runs the identical single-batch kernel.

Per-core dataflow (S=2048, D=512, H=8, HD=64):
  phase 0: PE-transpose src, in_proj_w, out_w into contraction-major
           layouts (srcT [D,S], winT [D,3D], woT [D,D]).
  phase 1: qkvT [3D, S] = winT.T @ srcT on PE (fp32r), bias added on DVE
           during PSUM evacuation.  q/k kept fp32r, v cast to bf16.
  phase 2: per s-chunk of 512, per head:
           scoresT [t,s] on PE (K=64, head pairs placed at partition
           offsets 0/64 so row-tiled matmuls overlap);
           exp on ACT straight out of PSUM (scale=1/8 fused) -> bf16;
           ctx + rowsum via one PE matmul against [v | 1];
           attn-average path: PE transposes exp tiles back to [s,t] and a
           DVE scalar_tensor_tensor accumulates c_h[s]*exp into fp16;
           out = ctxT.T @ woT + bias on PE/DVE.
"""

import numpy as np

S, B, D, H = 2048, 8, 512, 8
HD = D // H  # 64

_CACHE = {}


def _build_nc(qk_dtype_name: str):
    import concourse.bass as bass  # noqa: F401
    import concourse.tile as tile
    from concourse import bacc, mybir
    from concourse.masks import make_identity

    F32 = mybir.dt.float32
    F32R = mybir.dt.float32r
    BF16 = mybir.dt.bfloat16
    FP16 = mybir.dt.float16
    QK = {"f32r": F32R, "bf16": BF16}[qk_dtype_name]
    AF = mybir.ActivationFunctionType
    OP = mybir.AluOpType

    nc = bacc.Bacc("TRN2", target_bir_lowering=False, debug=False, num_devices=B)

    src = nc.dram_tensor("src", [S, D], F32, kind="ExternalInput")
    win = nc.dram_tensor("in_proj_w", [3 * D, D], F32, kind="ExternalInput")
    bin_ = nc.dram_tensor("in_proj_b", [3 * D], F32, kind="ExternalInput")
    wo = nc.dram_tensor("out_w", [D, D], F32, kind="ExternalInput")
    bo = nc.dram_tensor("out_b", [D], F32, kind="ExternalInput")
    out = nc.dram_tensor("out", [S, D], F32, kind="ExternalOutput")
    attn = nc.dram_tensor("attn", [S, S], F32, kind="ExternalOutput")

    NS = S // 128  # 16 s/t tiles
    NC_ = 4        # s-chunks of 512
    ST_PER_C = 4   # s-tiles per chunk
    NT = 16        # t tiles

    with tile.TileContext(nc) as tc:
        with (
            tc.tile_pool(name="consts", bufs=1) as consts,
            tc.tile_pool(name="qk", bufs=1) as qk_pool,
            tc.tile_pool(name="vext", bufs=1) as vext_pool,
            tc.tile_pool(name="wacc", bufs=4) as wacc_pool,
            tc.tile_pool(name="persist", bufs=1) as persist,
        ):
            # ---------------- constants ----------------
            ident_bf = consts.tile([128, 128], BF16, tag="ident_bf")
            make_identity(nc, ident_bf)
            ident_f32 = consts.tile([128, 128], F32, tag="ident_f32")
            make_identity(nc, ident_f32)

            bin_sb = consts.tile([128, 12], F32, tag="bin")
            # in_proj_b[128*j + p] -> bin_sb[p, j]
            nc.gpsimd.dma_start(
                out=bin_sb,
                in_=bass.AP(tensor=bin_, offset=0, ap=[[1, 128], [128, 12]]),
            )
            bo_bcast = consts.tile([128, D], F32, tag="bo")
            nc.gpsimd.dma_start(
                out=bo_bcast,
                in_=bass.AP(tensor=bo, offset=0, ap=[[0, 128], [1, D]]),
            )

            # persistent activations
            qkT = qk_pool.tile([128, 8, S], QK, tag="qkT")      # q rows 0-511, k rows 512-1023
            vext = vext_pool.tile([128, H, NT, HD + 1], BF16, tag="vext")
            ctx_sb = persist.tile([128, ST_PER_C, D], BF16, tag="ctx")
            ctxT_sb = persist.tile([128, 4, 512], BF16, tag="ctxT")
            woT = persist.tile([128, 4, D], BF16, tag="woT")

            ps_all_cm = tc.tile_pool(name="ps_all", bufs=2, space="PSUM")
            ps_all = ps_all_cm.__enter__()

            # ---------------- phase 0: transposes ----------------
            with (
                tc.tile_pool(name="stage", bufs=2) as stage_pool,
                tc.tile_pool(name="stageT", bufs=1) as stageT_pool,
            ):
                srcT = stageT_pool.tile([128, 4, S], F32R, tag="srcT")
                winT = stageT_pool.tile([128, 4, 3 * D], F32R, tag="winT")
                vT = stageT_pool.tile([128, 4, S], BF16, tag="vT")

                # src [S, D] -> srcT[p, kk, s] = src[s, 128*kk + p]
                for stg in range(4):  # groups of 4 s-tiles
                    st_sb = stage_pool.tile([128, 4, D], F32, tag="stage")
                    nc.sync.dma_start(
                        out=st_sb, in_=src[128 * 4 * stg : 128 * 4 * (stg + 1), :]
                        .rearrange("(a p) d -> p a d", p=128)
                    )
                    for kk in range(4):
                        pt = ps_all.tile([128, 512], F32, tag="ctxp")
                        for j in range(4):
                            nc.tensor.transpose(
                                pt[:, 128 * j : 128 * (j + 1)],
                                st_sb[:, j, 128 * kk : 128 * (kk + 1)],
                                ident_f32,
                            )
                        nc.scalar.copy(
                            srcT[:, kk, 512 * stg : 512 * (stg + 1)], pt
                        )

                # win [1536, 512] -> winT[p, kk, dd] = win[dd, 128*kk + p]
                for jg in range(3):  # groups of 4 dd-tiles
                    st_sb = stage_pool.tile([128, 4, D], F32, tag="stage")
                    nc.sync.dma_start(
                        out=st_sb, in_=win[128 * 4 * jg : 128 * 4 * (jg + 1), :]
                        .rearrange("(a p) d -> p a d", p=128)
                    )
                    for kk in range(4):
                        pt = ps_all.tile([128, 512], F32, tag="ctxp")
                        for j in range(4):
                            nc.tensor.transpose(
                                pt[:, 128 * j : 128 * (j + 1)],
                                st_sb[:, j, 128 * kk : 128 * (kk + 1)],
                                ident_f32,
                            )
                        nc.scalar.copy(
                            winT[:, kk, 512 * jg : 512 * (jg + 1)], pt
                        )

                # wo [512, 512] -> woT[p, kk, d'] = wo[d', 128*kk + p]  (bf16)
                st_sb = stage_pool.tile([128, 4, D], F32, tag="stage")
                nc.sync.dma_start(
                    out=st_sb, in_=wo[:, :].rearrange("(a p) d -> p a d", p=128)
                )
                for kk in range(4):
                    pt = ps_all.tile([128, 512], F32, tag="ctxp")
                    for j in range(4):
                        nc.tensor.transpose(
                            pt[:, 128 * j : 128 * (j + 1)],
                            st_sb[:, j, 128 * kk : 128 * (kk + 1)],
                            ident_f32,
                        )
                    nc.scalar.copy(woT[:, kk, :], pt)

                # vext = [v | 1] per head, emitted as soon as its vT rows land
                nc.vector.memset(vext[:, :, :, HD : HD + 1], 1.0)

                def emit_vext_pair(a):
                    for h in (2 * a, 2 * a + 1):
                        p0 = 64 * (h % 2)
                        jj = h // 2
                        for tg in range(4):  # groups of 4 t-tiles
                            pv = ps_all.tile(
                                [128, 8, 128], BF16, tag="wp", name=f"pv_{h}_{tg}"
                            )
                            for j in range(4):
                                tt = 4 * tg + j
                                nc.tensor.transpose(
                                    pv[:, j, 0:HD],
                                    vT[p0 : p0 + 64, jj, 128 * tt : 128 * (tt + 1)],
                                    ident_bf[p0 : p0 + 64, p0 : p0 + 64],
                                )
                            nc.scalar.copy(
                                vext[:, h, 4 * tg : 4 * (tg + 1), 0:HD],
                                pv[:, 0:4, 0:HD],
                            )


                # ---------------- phase 1: qkvT ----------------
                J_ORDER = [4, 0, 8, 5, 1, 9, 6, 2, 10, 7, 3, 11]
                for j in J_ORDER:
                    for sc in range(4):
                        pq = ps_all.tile([128, 512], F32, tag="ctxp")
                        for kk in range(4):
                            nc.tensor.matmul(
                                pq,
                                winT[:, kk, 128 * j : 128 * (j + 1)],
                                srcT[:, kk, 512 * sc : 512 * (sc + 1)],
                                start=(kk == 0),
                                stop=(kk == 3),
                            )
                        if j < 8:
                            dst = qkT[:, j, 512 * sc : 512 * (sc + 1)]
                        else:
                            dst = vT[:, j - 8, 512 * sc : 512 * (sc + 1)]
                        nc.scalar.activation(
                            dst, pq, AF.Identity,
                            bias=bin_sb[:, j : j + 1], scale=1.0,
                        )
                    if j >= 8:
                        emit_vext_pair(j - 8)

            # ---------------- phase 2: attention ----------------
            with (
                tc.tile_pool(name="expT", bufs=3) as exp_pool,
                tc.tile_pool(name="small", bufs=8) as small,
                tc.tile_pool(name="wstage", bufs=5) as wstage_pool,
                tc.tile_pool(name="ostage", bufs=2) as ostage_pool,
            ):
                sc_ps = ctx_ps = w_ps = ps_all
                for c in range(NC_):  # s-chunk of 512
                    accs = []
                    wfs = []
                    for st in range(ST_PER_C):
                        acc_t = wacc_pool.tile([128, S], FP16, tag="acc", name=f"acc_{c}_{st}")
                        accs.append(acc_t)
                        wf_t = wstage_pool.tile([128, S], F32, tag="wstage", name=f"wf_{c}_{st}")
                        wfs.append(wf_t)
                    exps = {}

                    def emit_scores_exp(h, c=c, exps=None):
                        p0 = 64 * (h % 2)
                        jq = h // 2
                        jk = 4 + h // 2
                        eT = exps[h]
                        for g in range(NT // 2):
                            scp = sc_ps.tile(
                                [128, 2, 512], F32, tag="scp", name=f"scp_{c}_{h}_{g}"
                            )
                            for i in range(2):
                                tt = 2 * g + i
                                nc.tensor.matmul(
                                    scp[:, i, :],
                                    qkT[p0 : p0 + 64, jk, 128 * tt : 128 * (tt + 1)],
                                    qkT[p0 : p0 + 64, jq, 512 * c : 512 * (c + 1)],
                                    start=True,
                                    stop=True,
                                )
                            nc.scalar.activation(
                                eT[:, 2 * g : 2 * (g + 1), :],
                                scp,
                                AF.Exp,
                                bias=0.0,
                                scale=float(HD) ** -0.5,
                            )

                    def emit_ctx_w(h, c=c, accs=None, wfs=None, exps=None):
                        eT = exps[h]
                        for st in range(ST_PER_C):
                            s_lo = 128 * st
                            ctxp = ctx_ps.tile(
                                [128, 512], F32, tag="ctxp", name=f"ctxp_{c}_{h}_{st}"
                            )
                            for tt in range(NT):
                                nc.tensor.matmul(
                                    ctxp[:, 0 : HD + 1],
                                    eT[:, tt, s_lo : s_lo + 128],
                                    vext[:, h, tt, :],
                                    start=(tt == 0),
                                    stop=(tt == NT - 1),
                                )
                            r = small.tile([128, 1], F32, tag="recip", name=f"r_{c}_{h}_{st}")
                            nc.vector.reciprocal(r, ctxp[:, HD : HD + 1])
                            ws = small.tile([128, 1], F32, tag="wscale", name=f"ws_{c}_{h}_{st}")
                            nc.vector.tensor_scalar_mul(ws, r, 1.0 / H)
                            nc.vector.tensor_scalar(
                                ctx_sb[:, st, HD * h : HD * (h + 1)],
                                ctxp[:, 0:HD],
                                r[:, 0:1],
                                None,
                                OP.mult,
                            )
                            # transpose exp back to [s, t'] and accumulate
                            for half in range(2):
                                wp = w_ps.tile(
                                    [128, 8, 128], BF16, tag="wp",
                                    name=f"wp_{c}_{h}_{st}_{half}",
                                )
                                for j in range(8):
                                    tt = 8 * half + j
                                    nc.tensor.transpose(
                                        wp[:, j, :],
                                        eT[:, tt, s_lo : s_lo + 128],
                                        ident_bf,
                                    )
                                acc_sl = accs[st][:, 1024 * half : 1024 * (half + 1)]
                                wp_flat = wp.rearrange("p a b -> p (a b)")
                                if h == 0:
                                    nc.vector.tensor_scalar(
                                        acc_sl, wp_flat, ws[:, 0:1], None, OP.mult
                                    )
                                elif h == H - 1:
                                    wf_sl = wfs[st][:, 1024 * half : 1024 * (half + 1)]
                                    nc.vector.scalar_tensor_tensor(
                                        wf_sl, wp_flat, ws[:, 0:1], acc_sl,
                                        OP.mult, OP.add,
                                    )
                                else:
                                    nc.vector.scalar_tensor_tensor(
                                        acc_sl, wp_flat, ws[:, 0:1], acc_sl,
                                        OP.mult, OP.add,
                                    )

                    # head-granular software pipeline: scores/exp of head k
                    # overlap ctx + attn-average of head k-1
                    for k in range(H + 1):
                        if k < H:
                            exps[k] = exp_pool.tile(
                                [128, NT, 512], BF16, tag="expT", name=f"expT_{c}_{k}"
                            )
                            emit_scores_exp(k, exps=exps)
                        if k > 0:
                            emit_ctx_w(k - 1, accs=accs, wfs=wfs, exps=exps)

                    # finalize chunk: attn rows out, ctxT, out-proj
                    for st in range(ST_PER_C):
                        nc.sync.dma_start(
                            out=attn[128 * (ST_PER_C * c + st) : 128 * (ST_PER_C * c + st + 1), :],
                            in_=wfs[st],
                        )
                        # ctxT: [s, d] -> [d, s] for out-proj lhsT
                        wp = w_ps.tile([128, 8, 128], BF16, tag="wp", name=f"ctxT_{c}_{st}")
                        for dt in range(4):
                            nc.tensor.transpose(
                                wp[:, dt, :],
                                ctx_sb[:, st, 128 * dt : 128 * (dt + 1)],
                                ident_bf,
                            )
                        nc.vector.tensor_copy(
                            ctxT_sb[:, :, 128 * st : 128 * (st + 1)],
                            wp[:, 0:4, :],
                        )
                    for st in range(ST_PER_C):
                        outp = ctx_ps.tile(
                            [128, 512], F32, tag="ctxp", name=f"outp_{c}_{st}"
                        )
                        for dt in range(4):
                            nc.tensor.matmul(
                                outp,
                                ctxT_sb[:, dt, 128 * st : 128 * (st + 1)],
                                woT[:, dt, :],
                                start=(dt == 0),
                                stop=(dt == 3),
                            )
                        ost = ostage_pool.tile(
                            [128, D], F32, tag="ostage", name=f"ost_{c}_{st}"
                        )
                        nc.vector.tensor_tensor(ost, outp, bo_bcast, OP.add)
                        nc.sync.dma_start(
                            out=out[128 * (ST_PER_C * c + st) : 128 * (ST_PER_C * c + st + 1), :],
                            in_=ost,
                        )

            ps_all_cm.__exit__(None, None, None)

    nc.compile()
    return nc


def _get_nc(qk_dtype_name="f32r"):
    key = qk_dtype_name
    if key not in _CACHE:
        _CACHE[key] = _build_nc(qk_dtype_name)
    return _CACHE[key]


def kernel(src, in_proj_w, in_proj_b, out_w, out_b):
    from concourse.bass_utils import run_bass_kernel_spmd

    src = np.ascontiguousarray(np.asarray(src, dtype=np.float32))
    in_proj_w = np.ascontiguousarray(np.asarray(in_proj_w, dtype=np.float32))
    in_proj_b = np.ascontiguousarray(np.asarray(in_proj_b, dtype=np.float32))
    out_w = np.ascontiguousarray(np.asarray(out_w, dtype=np.float32))
    out_b = np.ascontiguousarray(np.asarray(out_b, dtype=np.float32))

    nc = _get_nc()
    in_maps = [
        {
            "src": np.ascontiguousarray(src[:, b, :]),
            "in_proj_w": in_proj_w,
            "in_proj_b": in_proj_b,
            "out_w": out_w,
            "out_b": out_b,
        }
        for b in range(B)
    ]
    res = run_bass_kernel_spmd(nc, in_maps, core_ids=list(range(B)))
    out = np.empty((S, B, D), dtype=np.float32)
    attn = np.empty((B, S, S), dtype=np.float32)
    for b in range(B):
        out[:, b, :] = res.results[b]["out"]
        attn[b] = res.results[b]["attn"]
    return out, attn


if __name__ == "__main__":
    rng = np.random.default_rng(0)
    ins = {
        "src": rng.standard_normal((S, B, D), dtype=np.float32),
        "in_proj_w": rng.standard_normal((3 * D, D), dtype=np.float32) * D**-0.5,
        "in_proj_b": rng.standard_normal((3 * D,), dtype=np.float32) * 0.02,
        "out_w": rng.standard_normal((D, D), dtype=np.float32) * D**-0.5,
        "out_b": rng.standard_normal((D,), dtype=np.float32) * 0.02,
    }
    o, a = kernel(**ins)
    print("out", o.shape, "attn", a.shape)
